# revision 11
# baseline (speedup 1.0000x reference)
"""Trainium2 Bass kernel for 2-layer bidirectional GRU (B=64, T=256, D=512, H=512).

Sharding: data-parallel over batch, B=8 per core on 8 NeuronCores; weights replicated.
Layout strategy: everything transposed (H on partitions, batch on free dim).
 - Input GEMMs xg = Wi_aug @ [x; 1] batched over (t, b) columns, biases folded in
   via an augmented constant-1 contraction row.  bf16 operands, f32 PSUM.
 - Scan: per step, gh^T = Wh^T-tiles (stationary, bf16) @ h^T (moving, [128, 8]).
   Gates computed in f32 on DVE/ACT; h' cast to bf16 directly into the y-chunk
   staging tile which doubles as next step's matmul rhs.
 - Backward cells run the same code with compile-time reversed chunk/column
   indexing, so all HBM tensors stay in forward time order.
"""
import os
import numpy as np
import ml_dtypes

import concourse.bass as bass
import concourse.mybir as mybir
import concourse.tile as tile
from concourse import bacc
from concourse.bass_utils import run_bass_kernel_spmd

BF16 = ml_dtypes.bfloat16
F32 = np.float32
NCORES = 8
Bc = 8          # batch per core
H = 512
D = 512
KT = 4          # k-tiles for H contraction (512/128)
MT = 12         # m-tiles of gate dim 1536

LAST_RESULT = None  # stashed BassKernelResults for test harness


def _bf(x):
    return np.ascontiguousarray(x, dtype=BF16)


# ----------------------------------------------------------------- device code
def build_nc(T, Tc):
    NCH = T // Tc
    NTB = T * Bc          # (t, b) columns
    NW = min(512, NTB)    # GEMM n-chunk width
    NNC = NTB // NW       # n-chunks in GEMMs
    f32 = mybir.dt.float32
    bf16 = mybir.dt.bfloat16
    SIG = mybir.ActivationFunctionType.Sigmoid
    TANH = mybir.ActivationFunctionType.Tanh

    nc = bacc.Bacc()
    # inputs
    xt0 = nc.dram_tensor("xt0", [128, 5, NTB], bf16, kind="ExternalInput")
    wi0 = {c: nc.dram_tensor(f"wi0{c}", [128, 5, 1536], bf16, kind="ExternalInput") for c in "fb"}
    wh0 = {c: nc.dram_tensor(f"wh0{c}", [128, 4, 1536], bf16, kind="ExternalInput") for c in "fb"}
    wi1 = {c: nc.dram_tensor(f"wi1{c}", [128, 9, 1536], bf16, kind="ExternalInput") for c in "fb"}
    wh1 = {c: nc.dram_tensor(f"wh1{c}", [128, 4, 1536], bf16, kind="ExternalInput") for c in "fb"}
    bhn = {(l, c): nc.dram_tensor(f"bhn{l}{c}", [128, 4, Bc], f32, kind="ExternalInput")
           for l in (0, 1) for c in "fb"}
    ones1 = nc.dram_tensor("ones1", [128, 512], bf16, kind="ExternalInput")
    # internal scratch
    xg = {(l, c): nc.dram_tensor(f"xg{l}{c}", [128, MT, NTB], bf16, kind="Internal")
          for l in (0, 1) for c in "fb"}
    y0 = {c: nc.dram_tensor(f"y0{c}", [128, 4, NTB], bf16, kind="Internal") for c in "fb"}
    # outputs
    y1 = {c: nc.dram_tensor(f"y1{c}", [128, 4, NTB], bf16, kind="ExternalOutput") for c in "fb"}
    fin = nc.dram_tensor("fin", [4, 128, KT, Bc], f32, kind="ExternalOutput")

    with tile.TileContext(nc) as tc:
        with (
            tc.tile_pool(name="small", bufs=1) as smallp,
            tc.tile_pool(name="gates", bufs=3) as gp,
            tc.tile_pool(name="state", bufs=1) as statep,
            tc.tile_pool(name="gich", bufs=2) as gich,
            tc.tile_pool(name="ych", bufs=2) as ych,
            tc.tile_pool(name="evac", bufs=3) as evacp,
            tc.tile_pool(name="y0ch", bufs=2) as y0chp,
            tc.tile_pool(name="gps", bufs=2, space="PSUM") as gps,
            tc.tile_pool(name="sps", bufs=3, space="PSUM") as sps,
        ):
            bhn_t = {k: smallp.tile_from(v[:], name=f"bhn{k[0]}{k[1]}") for k, v in bhn.items()}
            zeros = smallp.tile([128, 4, Bc], bf16, tag="zeros")
            nc.vector.memset(zeros[:], 0.0)
            ones1_t = smallp.tile_from(ones1[:], name="ones1t")

            def gemm(wi_t, nkt, rhs_fn, xg_dram):
                """xg[:, m, nch*512:...] = sum_k wi_t[:,k,m*128:...]^T @ rhs_fn(k, nch)"""
                for nch in range(NNC):
                    rhs = rhs_fn(nch)
                    for m in range(MT):
                        ps = gps.tile([128, NW], f32, tag="gps")
                        for k in range(nkt):
                            nc.tensor.matmul(
                                ps[:, :], wi_t[:, k, m * 128:(m + 1) * 128],
                                rhs(k), start=(k == 0), stop=(k == nkt - 1))
                        ev = evacp.tile([128, NW], bf16, tag="ev")
                        if m % 2 == 0:
                            nc.vector.tensor_copy(ev[:], ps[:])
                        else:
                            nc.scalar.copy(ev[:], ps[:])
                        nc.sync.dma_start(xg_dram[:, m, nch * NW:(nch + 1) * NW], ev[:])

            class CellState:
                pass

            def make_state(cell):
                st = CellState()
                st.h = statep.tile([128, 4, Bc], f32, tag=f"h{cell}")
                nc.vector.memset(st.h[:], 0.0)
                st.hsrc = (zeros, 0)   # (tile [128,4,*] bf16, col offset)
                st.gi = None
                st.yc = None
                return st

            def scan_step(st, s, wh_t, xg_dram, y_dram, bhn_tile, fwd, cell, fin_idx):
                tci, ti = divmod(s, Tc)
                rc = tci if fwd else NCH - 1 - tci       # chunk index in dram
                cc = ti if fwd else Tc - 1 - ti          # column within chunk
                if ti == 0:
                    st.gi = gich.tile([128, MT, Tc * Bc], bf16, tag=f"gi{cell}")
                    nc.sync.dma_start(st.gi[:], xg_dram[:, :, rc * Tc * Bc:(rc + 1) * Tc * Bc])
                    st.yc = ych.tile([128, 4, Tc * Bc], bf16, tag=f"yc{cell}")
                gsl = slice(cc * Bc, (cc + 1) * Bc)
                hs, ho = st.hsrc
                ps_rz = sps.tile([128, 8, Bc], f32, tag="psrz")
                ps_n = sps.tile([128, 4, Bc], f32, tag="psn")
                for m in range(8):
                    for k in range(KT):
                        nc.tensor.matmul(
                            ps_rz[:, m, :], wh_t[:, k, m * 128:(m + 1) * 128],
                            hs[:, k, ho:ho + Bc], start=(k == 0), stop=(k == KT - 1))
                for m in range(4):
                    for k in range(KT):
                        nc.tensor.matmul(
                            ps_n[:, m, :], wh_t[:, k, (8 + m) * 128:(9 + m) * 128],
                            hs[:, k, ho:ho + Bc], start=(k == 0), stop=(k == KT - 1))
                grz = gp.tile([128, 8, Bc], f32, tag="grz")
                nc.vector.tensor_add(grz[:], ps_rz[:], st.gi[:, 0:8, gsl])
                rz = gp.tile([128, 8, Bc], f32, tag="rz")
                nc.scalar.activation(rz[:], grz[:], SIG)
                v = gp.tile([128, 4, Bc], f32, tag="v")
                nc.vector.tensor_add(v[:], ps_n[:], bhn_tile[:])
                u = gp.tile([128, 4, Bc], f32, tag="u")
                nc.vector.tensor_mul(u[:], rz[:, 0:4, :], v[:])
                w = gp.tile([128, 4, Bc], f32, tag="w")
                nc.vector.tensor_add(w[:], u[:], st.gi[:, 8:12, gsl])
                nt = gp.tile([128, 4, Bc], f32, tag="nt")
                nc.scalar.activation(nt[:], w[:], TANH)
                d = gp.tile([128, 4, Bc], f32, tag="d")
                nc.vector.tensor_sub(d[:], st.h[:], nt[:])
                e = gp.tile([128, 4, Bc], f32, tag="e")
                nc.vector.tensor_mul(e[:], rz[:, 4:8, :], d[:])
                nc.vector.tensor_add(st.h[:], nt[:], e[:])
                nc.vector.tensor_copy(st.yc[:, :, cc * Bc:(cc + 1) * Bc], st.h[:])
                st.hsrc = (st.yc, cc * Bc)
                if ti == Tc - 1:
                    nc.sync.dma_start(
                        y_dram[:, :, rc * Tc * Bc:(rc + 1) * Tc * Bc], st.yc[:])
                if s == T - 1:
                    nc.sync.dma_start(fin[fin_idx], st.h[:])

            # ---------------- phase 0
            with tc.tile_pool(name="w0", bufs=1) as w0p:
                xt0_t = w0p.tile_from(xt0[:], name="xt0t")
                wi0_t = {c: w0p.tile_from(wi0[c][:], name=f"wi0t{c}") for c in "fb"}
                wh0_t = {c: w0p.tile_from(wh0[c][:], name=f"wh0t{c}") for c in "fb"}
                for c in "fb":
                    gemm(wi0_t[c], 5, lambda nch: (lambda k: xt0_t[:, k, nch * NW:(nch + 1) * NW]),
                         xg[(0, c)])
                sts = {c: make_state(c) for c in "fb"}
                for s in range(T):
                    scan_step(sts["f"], s, wh0_t["f"], xg[(0, "f")], y0["f"],
                              bhn_t[(0, "f")], True, "f", 0)
                    scan_step(sts["b"], s, wh0_t["b"], xg[(0, "b")], y0["b"],
                              bhn_t[(0, "b")], False, "b", 1)

            # ---------------- phase 1
            with tc.tile_pool(name="w1", bufs=1) as w1p:
                wi1_t = {c: w1p.tile_from(wi1[c][:], name=f"wi1t{c}") for c in "fb"}
                wh1_t = {c: w1p.tile_from(wh1[c][:], name=f"wh1t{c}") for c in "fb"}

                def rhs1(nch):
                    yf = y0chp.tile([128, 4, NW], bf16, tag="y0f")
                    yb = y0chp.tile([128, 4, NW], bf16, tag="y0b")
                    nc.sync.dma_start(yf[:], y0["f"][:, :, nch * NW:(nch + 1) * NW])
                    nc.sync.dma_start(yb[:], y0["b"][:, :, nch * NW:(nch + 1) * NW])

                    def rhs(k):
                        if k < 4:
                            return yf[:, k, :]
                        if k < 8:
                            return yb[:, k - 4, :]
                        return ones1_t[:, :NW]
                    return rhs
                for c in "fb":
                    gemm(wi1_t[c], 9, rhs1, xg[(1, c)])
                sts = {c: make_state("1" + c) for c in "fb"}
                for s in range(T):
                    scan_step(sts["f"], s, wh1_t["f"], xg[(1, "f")], y1["f"],
                              bhn_t[(1, "f")], True, "1f", 2)
                    scan_step(sts["b"], s, wh1_t["b"], xg[(1, "b")], y1["b"],
                              bhn_t[(1, "b")], False, "1b", 3)
    nc.finalize()
    return nc


# ----------------------------------------------------------------- host side
def _pack_wi(Wi, bi, bh, nkt):
    """[128, nkt, 1536]: k-tiles of Wi^T plus bias row (bi+bh for r,z; bi for n)."""
    din = Wi.shape[1]
    bias = bi.astype(np.float64) + bh.astype(np.float64)
    bias[2 * H:] = bi[2 * H:]
    full = np.zeros((128, nkt, 1536), np.float32)
    wt = Wi.T.reshape(din // 128, 128, 1536)  # [kk, p, g]
    full[:, :din // 128] = wt.transpose(1, 0, 2)
    full[0, din // 128] = bias
    return _bf(full)


def _pack_wh(Wh):
    return _bf(Wh.T.reshape(4, 128, 1536).transpose(1, 0, 2))


def _pack_bhn(bh):
    return np.ascontiguousarray(
        np.repeat(bh[2 * H:].reshape(4, 128).T[:, :, None], Bc, axis=2), np.float32)


def kernel(inputs, l0f_Wi, l0f_Wh, l0f_bi, l0f_bh,
           l0b_Wi, l0b_Wh, l0b_bi, l0b_bh,
           l1f_Wi, l1f_Wh, l1f_bi, l1f_bh,
           l1b_Wi, l1b_Wh, l1b_bi, l1b_bh):
    global LAST_RESULT
    x = np.asarray(inputs, np.float32)
    B, T, _ = x.shape
    assert B == 64
    Tc = 32 if T % 32 == 0 else T
    nc = build_nc(T, Tc)

    shared = {
        "wi0f": _pack_wi(l0f_Wi, l0f_bi, l0f_bh, 5),
        "wi0b": _pack_wi(l0b_Wi, l0b_bi, l0b_bh, 5),
        "wi1f": _pack_wi(l1f_Wi, l1f_bi, l1f_bh, 9),
        "wi1b": _pack_wi(l1b_Wi, l1b_bi, l1b_bh, 9),
        "wh0f": _pack_wh(l0f_Wh), "wh0b": _pack_wh(l0b_Wh),
        "wh1f": _pack_wh(l1f_Wh), "wh1b": _pack_wh(l1b_Wh),
        "bhn0f": _pack_bhn(l0f_bh), "bhn0b": _pack_bhn(l0b_bh),
        "bhn1f": _pack_bhn(l1f_bh), "bhn1b": _pack_bhn(l1b_bh),
        "ones1": np.ones((128, 512), BF16),
    }
    # xt0 per core: [128, 5, T*Bc], cols (t, b)
    xr = x.reshape(NCORES, Bc, T, 4, 128)  # [c, b, t, kk, p]
    in_maps = []
    for c in range(NCORES):
        xt = np.zeros((128, 5, T * Bc), np.float32)
        xt[:, :4] = xr[c].transpose(3, 2, 1, 0).reshape(128, 4, T * Bc)
        xt[0, 4] = 1.0
        in_maps.append({**shared, "xt0": _bf(xt)})

    res = run_bass_kernel_spmd(nc, in_maps, core_ids=list(range(NCORES)),
                               trace=bool(int(os.environ.get("GRU_TRACE", "0"))))
    LAST_RESULT = res

    out = np.zeros((B, T, 2 * H), np.float32)
    finals = np.zeros((4, B, H), np.float32)
    for c in range(NCORES):
        r = res.results[c]
        for ci, name in ((0, "y1f"), (1, "y1b")):
            a = np.asarray(r[name]).astype(np.float32)
            a = a.reshape(128, 4, T, Bc).transpose(3, 2, 1, 0).reshape(Bc, T, H)
            out[Bc * c:Bc * (c + 1), :, ci * H:(ci + 1) * H] = a
        f = np.asarray(r["fin"]).transpose(0, 3, 2, 1)
        finals[:, Bc * c:Bc * (c + 1), :] = f.reshape(4, Bc, H)
    return out, finals
